# revision 40
# baseline (speedup 1.0000x reference)
"""Trainium2 Bass kernel for CustomAttention (B=4, S=2048, D=1024, H=16).

Sharding: 8 cores = 4 batches x 2 head-halves (8 heads each). Each core
computes Q/K/V projections for its 512 head-dims, attention for its 8 heads
over all 2048 queries, and a partial out-projection (contraction over its 512
dims). Host sums the two partial outputs per batch; bo/2 is added on each core
so the host sum carries the full bias.

v2 structure (vs v1 baseline at 445us):
  - Paired-seg QK: each PSUM seg tile [P, 2, 3kti, QC] holds BOTH heads of a
    pair, so the two heads' K=64 matmuls (row groups 0-63 / 64-127 via
    tile_position) become ready at the same instant, sit adjacent in the PE
    queue, and run concurrently (2 rhs streams into disjoint row halves).
    Serial QK was ~109us of PE; paired ~60us.
  - pt is one h-major tile [P, 2, NKT, QC] bf16 per (qc,j) unit; exp writes
    both heads in one ACT call per seg (1536 elems: 3 banks, double buffered).
  - Prologue interleave: QK segs of the first 4 units are emitted between
    per-pair K-projection passes, so ScalarE exp starts ~15us in, not ~45us.
  - V projection and Q sc1-3 emitted as small sub-passes (one stg4 group /
    one pair at a time) spread between units so ACT is never starved by a
    monolithic 13.6us projection block.
  - exp in [P,2,3,QC] PSUM segs; SEGS kti split {3,3,3,3,3,1}.
  - All matmul operands 16-bit; softmax scale folded into Wq/bq host-side.
  - PV unchanged: per (qc,j): 2 serial head chains M=65 (v_pad ones column
    gives softmax denominators); one batched reciprocal per qc; normalization
    multiply on DVE; out-projection chains trickled between units.
  - mask / key_padding_mask are all-ones for this problem's inputs => identity;
    a numpy fallback handles the (never-hit) general case.
"""

import math

import numpy as np

B, S, D = 4, 2048, 1024
H, DH = 16, 64       # global heads
HL = 8               # local heads per core
P = 128
NPAIR = HL // 2      # 4 local head pairs
NKT = S // P         # 16 key tiles
QC = 256             # query chunk for attention
NQC = S // QC        # 8
DL = 512             # local projection width (8 heads x 64)
SCALE = math.log(D) / math.sqrt(DH)
SEGS = [(0, 3), (3, 3), (6, 3), (9, 3), (12, 3), (15, 1)]

_CACHE = {}


def _build_nc():
    import concourse.bass as bass
    import concourse.bacc as bacc
    import concourse.mybir as mybir
    import concourse.tile as tile
    from contextlib import ExitStack

    f32 = mybir.dt.float32
    f16 = mybir.dt.float16
    bf16 = mybir.dt.bfloat16
    EXP = mybir.ActivationFunctionType.Exp
    ADD = mybir.AluOpType.add
    MULT = mybir.AluOpType.mult

    nc = bacc.Bacc("TRN2", target_bir_lowering=False, debug=False, num_devices=8)

    queryT = nc.declare_dram_parameter("queryT", [D, S], f16, isOutput=False)
    keyT = nc.declare_dram_parameter("keyT", [D, S], f16, isOutput=False)
    valueT = nc.declare_dram_parameter("valueT", [D, S], f16, isOutput=False)
    WqT = nc.declare_dram_parameter("WqT", [D, DL], f16, isOutput=False)
    WkT = nc.declare_dram_parameter("WkT", [D, DL], f16, isOutput=False)
    WvT = nc.declare_dram_parameter("WvT", [D, DL], f16, isOutput=False)
    WoT = nc.declare_dram_parameter("WoT", [DL, D], bf16, isOutput=False)
    bq_d = nc.declare_dram_parameter("bq", [DL], f32, isOutput=False)
    bk_d = nc.declare_dram_parameter("bk", [DL], f32, isOutput=False)
    bv_d = nc.declare_dram_parameter("bv", [1, DL], f32, isOutput=False)
    bo_d = nc.declare_dram_parameter("bo_half", [D], f32, isOutput=False)
    sel_d = nc.declare_dram_parameter("selc", [8, NPAIR * P], bf16, isOutput=False)
    outT = nc.declare_dram_parameter("outT", [D, S], f32, isOutput=True)

    with ExitStack() as ctx:
        tc = ctx.enter_context(tile.TileContext(nc))
        persist = ctx.enter_context(tc.tile_pool(name="persist", bufs=1))
        wpool = ctx.enter_context(tc.tile_pool(name="wpool", bufs=1))
        in4k = ctx.enter_context(tc.tile_pool(name="in4k", bufs=3))
        ptp = ctx.enter_context(tc.tile_pool(name="ptp", bufs=5))
        stgp = ctx.enter_context(tc.tile_pool(name="stgp", bufs=2))
        ost = ctx.enter_context(tc.tile_pool(name="ost", bufs=2))
        dnp = ctx.enter_context(tc.tile_pool(name="dnp", bufs=2))
        rcp = ctx.enter_context(tc.tile_pool(name="rcp", bufs=1))
        et = ctx.enter_context(tc.tile_pool(name="et", bufs=2, space="PSUM"))
        pvp = ctx.enter_context(tc.tile_pool(name="pvp", bufs=1, space="PSUM"))
        acc = ctx.enter_context(tc.tile_pool(name="acc", bufs=1, space="PSUM"))
        dram = ctx.enter_context(tc.tile_pool(name="dram", bufs=2, space="DRAM"))

        kT = [
            persist.tile([P, S], f16, tag=f"kT{j}", name=f"kT{j}")
            for j in range(NPAIR)
        ]
        qt = [
            persist.tile([P, S], f16, tag=f"qt{j}", name=f"qt{j}")
            for j in range(NPAIR)
        ]
        attn = [
            persist.tile([P, S], bf16, tag=f"at{j}", name=f"at{j}")
            for j in range(NPAIR)
        ]
        v_pad = persist.tile([P, NKT, HL, DH + 1], bf16, tag="v_pad")
        # sel[j]: [8, 128] 0/1 matrix; sel_j.T @ rcq broadcasts pair j's two
        # reciprocal rows (2j -> partitions 0-63, 2j+1 -> 64-127) in one MM
        sel = persist.tile([8, NPAIR, P], bf16, tag="sel")
        bq_sb = persist.tile([P, NPAIR], f32, tag="bq")
        bk_sb = persist.tile([P, NPAIR], f32, tag="bk")
        bo_sb = persist.tile([P, 8], f32, tag="bo")
        bv_bc = persist.tile([P, DL], f32, tag="bv_bc")

        # --- setup: wk + first input slice go down the DMA queue first so the
        # first projection chain starts ASAP; biases follow (not needed until
        # after the first 8-MM chain).
        wk = wpool.tile([P, 8, DL], f16, tag="w", name="wk")
        nc.sync.dma_start(wk[:], WkT[:].rearrange("(k p) c -> p k c", p=P))
        wq = wpool.tile([P, 8, DL], f16, tag="w2", name="wq")
        wv = wpool.tile([P, 8, DL], f16, tag="w3", name="wv")
        # wo reuses wk's slot (wk is dead after the last K-proj pass)
        wo = wpool.tile([P, NPAIR, D], bf16, tag="w", name="wo")

        def setup_rest():
            nc.sync.dma_start(bq_sb[:], bq_d.rearrange("(o p) -> p o", p=P))
            nc.sync.dma_start(bk_sb[:], bk_d.rearrange("(o p) -> p o", p=P))
            nc.sync.dma_start(bo_sb[:], bo_d.rearrange("(o p) -> p o", p=P))
            nc.sync.dma_start(bv_bc[:], bv_d[:].to_broadcast([P, DL]))
            # only the ones-columns need presetting; v dims are overwritten
            nc.vector.memset(v_pad[:, :, :, DH:DH + 1], 1.0)
            nc.sync.dma_start(sel[:], sel_d[:].rearrange("r (j p) -> r j p", j=NPAIR))

        def chunk(srcT, sc, name):
            # one batched DMA stages a full [1024, 512] input slice as
            # [P, 8kt, 512] (8 small DMAs would serialize ~650ns apiece on SP)
            t = in4k.tile([P, 8, 512], f16, tag="in", name=name)
            nc.sync.dma_start(
                t[:],
                srcT[:, sc * 512:(sc + 1) * 512].rearrange(
                    "(k p) c -> p k c", p=P
                ),
            )
            return t

        pi = 0
        steady = [False]  # True once PV chains own the pvp pool

        def proj_psum():
            nonlocal pi
            pool = acc if pi % 2 == 0 else pvp
            tg = "acc" if pi % 2 == 0 else "pv"
            t = pool.tile([P, 512], f32, tag=tg, name=f"pp{pi}")
            pi += 1
            return t

        # --- K/Q projection: per-(sc, pair) emission so passes interleave ---
        kq_chunks = {}

        def kq_fetch(which, sc):
            if which == "q" and sc == 0:
                nc.sync.dma_start(wq[:], WqT[:].rearrange("(k p) c -> p k c", p=P))
            srcT = keyT if which == "k" else queryT
            kq_chunks[(which, sc)] = chunk(srcT, sc, f"{which}c_{sc}")

        def kq_pair(which, sc, j):
            cs = kq_chunks[(which, sc)]
            w, dst, b_sb = (
                (wk, kT, bk_sb) if which == "k" else (wq, qt, bq_sb)
            )
            ps = proj_psum()
            for kt in range(8):
                nc.tensor.matmul(
                    out=ps[:],
                    lhsT=w[:, kt, j * P:(j + 1) * P],
                    rhs=cs[:, kt, :],
                    start=(kt == 0), stop=(kt == 7),
                )
            nc.vector.tensor_scalar_add(
                dst[j][:, sc * 512:(sc + 1) * 512], ps[:], b_sb[:, j:j + 1]
            )
            if j == NPAIR - 1:
                del kq_chunks[(which, sc)]

        # --- V projection: per-(g, stg4) sub-pass (8 chunk DMAs + 4 chains) ---
        def v_sub(g, stg4):
            if g == 0 and stg4 == 0:
                nc.sync.dma_start(wv[:], WvT[:].rearrange("(k p) c -> p k c", p=P))
            vc = chunk(valueT, stg4, f"vc{g}_{stg4}")
            for stl in range(4):
                st = stg4 * 4 + stl
                ps = proj_psum()
                for kt in range(8):
                    nc.tensor.matmul(
                        out=ps[:, 0:256],
                        lhsT=vc[:, kt, stl * P:(stl + 1) * P],
                        rhs=wv[:, kt, g * 256:(g + 1) * 256],
                        start=(kt == 0), stop=(kt == 7),
                    )
                nc.vector.tensor_tensor(
                    v_pad[:, st, g * 4:(g + 1) * 4, 0:DH],
                    ps[:, 0:256].rearrange("p (h d) -> p h d", h=4),
                    bv_bc[:, g * 256:(g + 1) * 256].rearrange(
                        "p (h d) -> p h d", h=4
                    ),
                    ADD,
                )

        # --- attention: paired-seg QK+exp, then (staggered) PV+norm ---
        pt_of = {}
        dnq_of = {}

        def att_qk_seg(qc, j, s):
            sb, sl = SEGS[s]
            if s == 0:
                pt_of[(qc, j)] = ptp.tile(
                    [P, 2, NKT, QC], bf16, tag="pt", name=f"pt{qc}_{j}"
                )
            pt = pt_of[(qc, j)]
            ett = et.tile([P, 2, 3, QC], f32, tag="et", name=f"et{qc}_{j}_{s}")
            for t_i in range(sl):
                kti = sb + t_i
                for h in range(2):
                    nc.tensor.matmul(
                        out=ett[:, h, t_i, :],
                        lhsT=kT[j][h * 64:(h + 1) * 64, kti * P:(kti + 1) * P],
                        rhs=qt[j][h * 64:(h + 1) * 64, qc * QC:(qc + 1) * QC],
                        start=True, stop=True,
                        tile_position=(h * 64, 0),
                    )
            if sl == 3:
                nc.scalar.activation(pt[:, :, sb:sb + 3, :], ett[:], EXP)
            else:
                nc.scalar.activation(
                    pt[:, :, sb:sb + sl, :], ett[:, :, 0:sl, :], EXP
                )

        pv_done = {qc: 0 for qc in range(NQC)}

        def att_pv(qc, j):
            if qc not in dnq_of:
                dnq_of[qc] = dnp.tile(
                    [2 * NPAIR, QC], f32, tag="dn", name=f"dn{qc}"
                )
            dnq = dnq_of[qc]
            pt = pt_of.pop((qc, j))
            pvt = pvp.tile([DH + 1, 2, QC], f32, tag="pv", name=f"pv{qc}_{j}")
            for h in range(2):
                for kti in range(NKT):
                    nc.tensor.matmul(
                        out=pvt[0:DH + 1, h, :],
                        lhsT=v_pad[:, kti, 2 * j + h, 0:DH + 1],
                        rhs=pt[:, h, kti, :],
                        start=(kti == 0), stop=(kti == NKT - 1),
                    )
            stg = stgp.tile([P, 2, QC], f32, tag="dnst")
            for h in range(2):
                nc.vector.tensor_copy(
                    out=attn[j][h * 64:(h + 1) * 64, qc * QC:(qc + 1) * QC],
                    in_=pvt[0:DH, h, :],
                )
            nc.vector.tensor_copy(
                out=stg[64:65, :, :], in_=pvt[DH:DH + 1, :, :]
            )
            nc.sync.dma_start(dnq[2 * j:2 * j + 2, :], stg[64:65, :, :])
            pv_done[qc] += 1
            if pv_done[qc] == NPAIR:
                norm(qc)
                o_proj_q(qc)

        def norm(qc):
            dnq = dnq_of.pop(qc)
            rcq = rcp.tile([2 * NPAIR, QC], bf16, tag="rc", name=f"rc{qc}")
            with nc.allow_low_precision(reason="1/denom broadcast via bf16 MM"):
                nc.vector.reciprocal(rcq[:], dnq[:])
            for jj in range(2):
                bc_ps = acc.tile([P, 2, QC], f32, tag="acc", name=f"bc{qc}_{jj}")
                for jl in range(2):
                    j = 2 * jj + jl
                    nc.tensor.matmul(
                        out=bc_ps[:, jl, :],
                        lhsT=sel[:, j, :],
                        rhs=rcq[:],
                        start=True, stop=True,
                    )
                for jl in range(2):
                    j = 2 * jj + jl
                    nc.vector.tensor_tensor(
                        attn[j][:, qc * QC:(qc + 1) * QC],
                        attn[j][:, qc * QC:(qc + 1) * QC],
                        bc_ps[:, jl, :],
                        MULT,
                    )

        o_chains = []
        drain_alt = [0]

        def o_chain(qc, c0, cw, dt, drain=False):
            # during the final drain (exp finished) alternate into the dead et
            # pool so consecutive chains don't serialize on the one acc bank
            if drain and drain_alt[0] % 2 == 1:
                ps = et.tile([P, 512], f32, tag="et", name=f"op{qc}_{dt}")
            else:
                ps = acc.tile([P, 512], f32, tag="acc", name=f"op{qc}_{dt}")
            drain_alt[0] += drain
            for ct in range(NPAIR):
                nc.tensor.matmul(
                    out=ps[:, 0:cw],
                    lhsT=wo[:, ct, dt * P:(dt + 1) * P],
                    rhs=attn[ct][:, c0:c0 + cw],
                    start=(ct == 0), stop=(ct == NPAIR - 1),
                )
            o_t = ost.tile([P, 512], f32, tag="ost")
            nc.vector.tensor_scalar_add(
                o_t[:, 0:cw], ps[:, 0:cw], bo_sb[:, dt:dt + 1]
            )
            nc.sync.dma_start(
                outT[dt * P:(dt + 1) * P, c0:c0 + cw], o_t[:, 0:cw]
            )

        def o_proj_q(qc):
            if qc == 0:
                nc.sync.dma_start(wo[:], WoT[:].rearrange("(c p) d -> p c d", p=P))
            if qc >= NQC - 2:
                c0, cw = qc * QC, QC       # last two qc: drain per-chunk
            elif qc % 2 == 1:
                c0, cw = (qc // 2) * 512, 512
            else:
                return
            for dt in range(8):
                o_chains.append((u_now[0] + 1, (qc, c0, cw, dt)))

        # --- emission -------------------------------------------------------
        # front 16 units: j0/j1 of qc0-3 first (only need K + Q sc0/sc1, all
        # staged in the prologue), then j2/j3; keeps the early steady slots
        # free of projection fills so ACT rebuilds backlog after the prologue.
        # qc4-7 keep all-4-j adjacency so their norms/out-proj don't pile into
        # the drain.
        units = (
            [(0, 0), (0, 1), (1, 0), (1, 1), (0, 2), (0, 3), (1, 2), (1, 3)]
            + [(qc, j) for qc in range(2, NQC) for j in range(NPAIR)]
        )
        n_units = len(units)
        u_now = [0]

        # prologue: K sc0 + Q sc0, then segs of units 0-3 interleaved with
        # the remaining K passes (seg s needs K sc: s0->0 s1->1 s2,s3->2
        # s4,s5->3), so exp starts ~15us in instead of ~45us.
        # prologue: each projection pair-chain (~0.9us) or v sub-pass (~3.4us)
        # is chased by one or two QK segs so ScalarE exp never runs dry while
        # the K/Q/V weights stream through PE. 5 units (pt bufs) in flight:
        # units 0-3 = (qc0/1, j0/1) fully, unit 4 = (0,2).
        kq_fetch("k", 0)
        setup_rest()
        for j in range(NPAIR):
            kq_pair("k", 0, j)
        kq_fetch("q", 0)
        for j in range(NPAIR):
            kq_pair("q", 0, j)
            if j < 2:
                att_qk_seg(0, j, 0)
                att_qk_seg(1, j, 0)
        att_qk_seg(0, 2, 0)
        kq_fetch("k", 1)
        for j in range(NPAIR):
            kq_pair("k", 1, j)
            if j < 2:
                att_qk_seg(0, j, 1)
                att_qk_seg(1, j, 1)
        att_qk_seg(0, 2, 1)
        kq_fetch("k", 2)
        for j in range(NPAIR):
            kq_pair("k", 2, j)
            if j < 2:
                att_qk_seg(0, j, 2)
                att_qk_seg(1, j, 2)
        att_qk_seg(0, 2, 2)
        kq_fetch("q", 1)
        kq_pair("q", 1, 0)
        att_qk_seg(0, 0, 3)
        att_qk_seg(1, 0, 3)
        kq_pair("q", 1, 1)
        att_qk_seg(0, 1, 3)
        att_qk_seg(1, 1, 3)
        kq_pair("q", 1, 2)
        att_qk_seg(0, 2, 3)
        kq_pair("q", 1, 3)
        kq_fetch("k", 3)
        kq_pair("k", 3, 0)
        att_qk_seg(0, 0, 4)
        att_qk_seg(1, 0, 4)
        kq_pair("k", 3, 1)
        att_qk_seg(0, 1, 4)
        att_qk_seg(1, 1, 4)
        kq_pair("k", 3, 2)
        att_qk_seg(0, 2, 4)
        kq_pair("k", 3, 3)
        v_sub(0, 0)
        att_qk_seg(0, 0, 5)
        att_qk_seg(1, 0, 5)
        v_sub(0, 1)
        att_qk_seg(0, 1, 5)
        att_qk_seg(1, 1, 5)
        v_sub(0, 2)
        att_qk_seg(0, 2, 5)
        v_sub(0, 3)
        steady[0] = True

        # steady loop: units 5..31. PV(u-5) drains at slot start (frees the pt
        # slot segs(u) needs; the 2-seg et runway covers ACT during the 3.4us
        # chain); fills spread 1/slot and interleaved between segs.
        LAG = 4
        fills = {
            6: [lambda: v_sub(1, 0)],
            7: [lambda: v_sub(1, 1), lambda: v_sub(1, 2)],
            8: [lambda: v_sub(1, 3)],
            9: [lambda: kq_fetch("q", 2), lambda: kq_pair("q", 2, 0)],
            10: [lambda: kq_pair("q", 2, 1), lambda: kq_pair("q", 2, 2)],
            11: [lambda: kq_pair("q", 2, 3)],
            16: [lambda: kq_fetch("q", 3), lambda: kq_pair("q", 3, 0)],
            17: [lambda: kq_pair("q", 3, 1), lambda: kq_pair("q", 3, 2)],
            18: [lambda: kq_pair("q", 3, 3)],
        }
        pv_next = 0
        for u in range(5, n_units):
            u_now[0] = u
            lag = 1 if u >= n_units - 4 else min(LAG, max(2, n_units - 1 - u))
            while pv_next <= u - lag:
                att_pv(*units[pv_next])
                pv_next += 1
            fl = list(fills.get(u, []))
            for s in range(6):
                att_qk_seg(*units[u], s)
                if s in (2, 4) and fl:
                    fl.pop(0)()
            for f in fl:
                f()
            npop = 3 if len(o_chains) > 12 else (2 if len(o_chains) > 4 else 1)
            for _ in range(npop):
                if o_chains and o_chains[0][0] <= u:
                    o_chain(*o_chains.pop(0)[1])
        u_now[0] = n_units + 2
        while pv_next < n_units:
            att_pv(*units[pv_next])
            pv_next += 1
            while o_chains:
                o_chain(*o_chains.pop(0)[1], drain=True)

    if not nc.is_finalized():
        nc.finalize()
    return nc


def get_nc():
    if "nc" not in _CACHE:
        _CACHE["nc"] = _build_nc()
    return _CACHE["nc"]


def make_in_maps(inputs):
    f16 = np.float16
    import ml_dtypes

    bf16 = ml_dtypes.bfloat16
    q = np.asarray(inputs["query"], np.float32)
    k = np.asarray(inputs["key"], np.float32)
    v = np.asarray(inputs["value"], np.float32)
    Wq = np.asarray(inputs["Wq"], np.float32) * SCALE  # fold softmax scale
    Wk = np.asarray(inputs["Wk"], np.float32)
    Wv = np.asarray(inputs["Wv"], np.float32)
    Wo = np.asarray(inputs["Wo"], np.float32)
    bq = np.asarray(inputs["bq"], np.float32) * SCALE
    bk = np.asarray(inputs["bk"], np.float32)
    bv = np.asarray(inputs["bv"], np.float32)
    bo_half = np.asarray(inputs["bo"], np.float32) * 0.5

    qT = [np.ascontiguousarray(q[b].T.astype(f16)) for b in range(B)]
    kTl = [np.ascontiguousarray(k[b].T.astype(f16)) for b in range(B)]
    vT = [np.ascontiguousarray(v[b].T.astype(f16)) for b in range(B)]
    WqTs = [np.ascontiguousarray(Wq.T[:, i * DL:(i + 1) * DL].astype(f16)) for i in range(2)]
    WkTs = [np.ascontiguousarray(Wk.T[:, i * DL:(i + 1) * DL].astype(f16)) for i in range(2)]
    WvTs = [np.ascontiguousarray(Wv.T[:, i * DL:(i + 1) * DL].astype(f16)) for i in range(2)]
    WoTs = [np.ascontiguousarray(Wo.T[i * DL:(i + 1) * DL, :].astype(bf16)) for i in range(2)]
    bqs = [np.ascontiguousarray(bq[i * DL:(i + 1) * DL]) for i in range(2)]
    bks = [np.ascontiguousarray(bk[i * DL:(i + 1) * DL]) for i in range(2)]
    bvs = [np.ascontiguousarray(bv[i * DL:(i + 1) * DL]).reshape(1, DL) for i in range(2)]

    selc = np.zeros((8, NPAIR, 128), np.float32)
    for j in range(NPAIR):
        selc[2 * j, j, 0:64] = 1.0
        selc[2 * j + 1, j, 64:128] = 1.0
    selc = np.ascontiguousarray(selc.reshape(8, NPAIR * 128).astype(bf16))

    in_maps = []
    for c in range(8):
        b, hh = c // 2, c % 2
        in_maps.append({
            "queryT": qT[b], "keyT": kTl[b], "valueT": vT[b],
            "WqT": WqTs[hh], "WkT": WkTs[hh], "WvT": WvTs[hh], "WoT": WoTs[hh],
            "bq": bqs[hh], "bk": bks[hh], "bv": bvs[hh], "bo_half": bo_half,
            "selc": selc,
        })
    return in_maps


def assemble(results):
    out = np.empty((B, S, D), np.float32)
    for b in range(B):
        out[b] = (results[2 * b]["outT"] + results[2 * b + 1]["outT"]).T
    return out


def _numpy_fallback(inputs):
    q = np.asarray(inputs["query"], np.float64)
    k = np.asarray(inputs["key"], np.float64)
    v = np.asarray(inputs["value"], np.float64)
    Wq, bq = np.asarray(inputs["Wq"], np.float64), np.asarray(inputs["bq"], np.float64)
    Wk, bk = np.asarray(inputs["Wk"], np.float64), np.asarray(inputs["bk"], np.float64)
    Wv, bv = np.asarray(inputs["Wv"], np.float64), np.asarray(inputs["bv"], np.float64)
    Wo, bo = np.asarray(inputs["Wo"], np.float64), np.asarray(inputs["bo"], np.float64)
    qp = (q @ Wq.T + bq).reshape(B, S, H, DH).transpose(0, 2, 1, 3)
    kp = (k @ Wk.T + bk).reshape(B, S, H, DH).transpose(0, 2, 1, 3)
    vp = (v @ Wv.T + bv).reshape(B, S, H, DH).transpose(0, 2, 1, 3)
    e = np.einsum("bhqd,bhkd->bhqk", qp, kp) * SCALE
    mask = np.asarray(inputs["mask"])
    kpm = np.asarray(inputs["key_padding_mask"])
    e = np.where(mask == 0, -np.inf, e)
    e = np.where(kpm[:, None, None, :] == 0, -np.inf, e)
    e -= e.max(axis=-1, keepdims=True)
    p = np.exp(e)
    p /= p.sum(axis=-1, keepdims=True)
    o = np.einsum("bhqk,bhkd->bhqd", p, vp).transpose(0, 2, 1, 3).reshape(B, S, D)
    return (o @ Wo.T + bo).astype(np.float32)


def kernel(**inputs):
    mask = np.asarray(inputs["mask"])
    kpm = np.asarray(inputs["key_padding_mask"])
    if not (mask.all() and kpm.all()):
        return _numpy_fallback(inputs)
    from concourse.bass_utils import run_bass_kernel_spmd

    nc = get_nc()
    in_maps = make_in_maps(inputs)
    res = run_bass_kernel_spmd(nc, in_maps, list(range(8)))
    return assemble(res.results)
